# revision 26
# baseline (speedup 1.0000x reference)
import numpy as np
import ml_dtypes

import concourse.bass as bass
import concourse.bacc as bacc
import concourse.mybir as mybir
from concourse.tile import TileContext
from concourse.bass_utils import run_bass_kernel_spmd

BF16 = ml_dtypes.bfloat16
FP8 = ml_dtypes.float8_e4m3
F32 = np.float32

B, H, W, D, K = 4, 384, 384, 16, 32
NCORES = 8
NPIX_TOT = B * H * W
NPIX = NPIX_TOT // NCORES
P = 128
TC = NPIX // P
NBANKS_A = 8

HT = TC // 2
CR = 48
WT = 16

PUSH_MARGIN = 0.25
PUSH_W = 1.0
PULL_W = 0.1
NCMP = K * (K - 1) / 2.0

CONSUMER_PATTERN = "APDPPPDPPDPPPDPPDP"

_built = {}


def _build_launch_a():
    nc = bacc.Bacc("TRN2", target_bir_lowering=False, debug=False)
    f8 = mybir.dt.float8e4
    f32 = mybir.dt.float32

    NGM = TC // 8
    embA = nc.dram_tensor("embA", [P, TC * D], f8, kind="ExternalInput")
    onehotA = nc.dram_tensor("onehotA", [P, TC * K], f8, kind="ExternalInput")
    outA = nc.dram_tensor("outA", [P, NBANKS_A, 8 * K], f32, kind="ExternalOutput")

    with TileContext(nc) as tc:
        with (
            tc.tile_pool(name="sbuf", bufs=1) as pool,
            tc.tile_pool(name="psum", bufs=1, space="PSUM") as psum_pool,
        ):
            emb_sb = pool.tile([P, TC * D], f8)
            onehot = pool.tile([P, TC, K], f8)

            NCH = 4
            ch = (TC * D) // NCH
            och = TC // NCH
            for i in range(NCH):
                nc.sync.dma_start(
                    out=emb_sb[:, i * ch : (i + 1) * ch],
                    in_=embA.ap()[:, i * ch : (i + 1) * ch],
                )
                nc.sync.dma_start(
                    out=onehot[:, i * och : (i + 1) * och, :],
                    in_=onehotA.ap().rearrange("p (t k) -> p t k", k=K)[
                        :, i * och : (i + 1) * och, :
                    ],
                )

            banks = [
                psum_pool.tile([P, 8, K], mybir.dt.float32, name=f"acc{b}")
                for b in range(NBANKS_A)
            ]
            for g in range(NGM):
                nc.tensor.matmul(
                    banks[g % NBANKS_A][:],
                    emb_sb[:, 128 * g : 128 * g + 128],
                    onehot[:, 8 * g : 8 * g + 8, :],
                    start=(g < NBANKS_A),
                    stop=(g >= NGM - NBANKS_A),
                )

            evac = pool.tile([P, NBANKS_A, 8 * K], f32)
            for b in range(NBANKS_A):
                if b % 2 == 0:
                    nc.scalar.copy(
                        out=evac[:, b, :],
                        in_=banks[b][:].rearrange("p a b -> p (a b)"),
                    )
                else:
                    nc.vector.tensor_copy(
                        out=evac[:, b, :],
                        in_=banks[b][:].rearrange("p a b -> p (a b)"),
                    )
            nc.sync.dma_start(out=outA.ap(), in_=evac[:])
    nc.compile()
    return nc


def _emit_sq_half(nc, dist, sq, pacc, h):
    HH = TC // 2
    sl = slice(h * HH, (h + 1) * HH)
    nc.vector.tensor_tensor(
        out=sq[:, sl, :], in0=dist[:, sl, :], in1=dist[:, sl, :],
        op=mybir.AluOpType.mult,
    )
    for b in range(4):
        nc.vector.tensor_reduce(
            out=pacc[:, h, b : b + 1],
            in_=sq[:, sl, b],
            axis=mybir.AxisListType.X,
            op=mybir.AluOpType.add,
        )


def _build_launch_b():
    nc = bacc.Bacc("TRN2", target_bir_lowering=False, debug=False)
    bf = mybir.dt.bfloat16
    f32 = mybir.dt.float32

    wev = nc.dram_tensor("wev", [CR, HT * P], bf, kind="ExternalInput")
    wod = nc.dram_tensor("wod", [CR, HT * P], bf, kind="ExternalInput")
    rtab = nc.dram_tensor("rtab", [CR, 4 * D], bf, kind="ExternalInput")
    pacc_d = nc.dram_tensor("pacc", [P, 8], f32, kind="ExternalOutput")

    AF = mybir.ActivationFunctionType

    with TileContext(nc) as tc:
        with (
            tc.tile_pool(name="sbuf", bufs=1) as pool,
            tc.tile_pool(name="work", bufs=3) as wpool,
            tc.tile_pool(name="psum", bufs=2, space="PSUM") as psum_pool,
        ):
            w_sb = pool.tile([P, HT, P], bf)
            rhs_sb = pool.tile([P, 4 * D], bf)
            dist = pool.tile([P, TC, 4], bf)
            sq = pool.tile([P, TC, 4], bf)
            pacc = pool.tile([P, 2, 4], f32)

            nc.sync.dma_start(out=rhs_sb[0:CR, :], in_=rtab.ap())
            nc.sync.dma_start(out=rhs_sb[64 : 64 + CR, :], in_=rtab.ap())
            bounds = [0, 32, 96, 160, 224, HT]
            for i in range(len(bounds) - 1):
                sl = slice(bounds[i], bounds[i + 1])
                nc.sync.dma_start(
                    out=w_sb[0:CR, sl, :],
                    in_=wev.ap().rearrange("r (t m) -> r t m", m=P)[:, sl, :],
                )
                nc.sync.dma_start(
                    out=w_sb[64 : 64 + CR, sl, :],
                    in_=wod.ap().rearrange("r (t m) -> r t m", m=P)[:, sl, :],
                )

            nwaves = HT // WT
            for w in range(nwaves):
                t0 = WT * w
                ps = psum_pool.tile(
                    [P, 2, WT, 4, D], mybir.dt.float32, tag="ps", name=f"ps_{w}"
                )
                for j in range(WT):
                    t = t0 + j
                    nc.tensor.matmul(
                        ps[:, 0, j, :, :].rearrange("p a b -> p (a b)"),
                        w_sb[0:CR, t, :],
                        rhs_sb[0:CR, :],
                        start=True,
                        stop=True,
                    )
                    nc.tensor.matmul(
                        ps[:, 1, j, :, :].rearrange("p a b -> p (a b)"),
                        w_sb[64 : 64 + CR, t, :],
                        rhs_sb[64 : 64 + CR, :],
                        start=True,
                        stop=True,
                    )
                kind = CONSUMER_PATTERN[w % len(CONSUMER_PATTERN)]
                out_ap = dist[:, 2 * t0 : 2 * t0 + 2 * WT, :].rearrange(
                    "p (h t) b -> p h t b", h=2
                )
                with nc.allow_low_precision("dist bf16; error averages out"):
                    if kind == "D":
                        nc.vector.tensor_reduce(
                            out=out_ap,
                            in_=ps[:],
                            axis=mybir.AxisListType.X,
                            op=mybir.AluOpType.add,
                            apply_absolute_value=True,
                        )
                    else:
                        absd = wpool.tile([P, 2, WT, 4, D], bf, tag="absd")
                        nc.scalar.activation(out=absd[:], in_=ps[:], func=AF.Abs)
                        h1 = wpool.tile([P, 2, WT, 4, D // 2], bf, tag="h1")
                        h1_eng = nc.gpsimd if kind == "P" else nc.vector
                        h1_eng.tensor_tensor(
                            out=h1[:],
                            in0=absd[:, :, :, :, 0 : D // 2],
                            in1=absd[:, :, :, :, D // 2 : D],
                            op=mybir.AluOpType.add,
                        )
                        h2 = wpool.tile([P, 2, WT, 4, D // 4], bf, tag="h2")
                        nc.vector.tensor_tensor(
                            out=h2[:],
                            in0=h1[:, :, :, :, 0 : D // 4],
                            in1=h1[:, :, :, :, D // 4 : D // 2],
                            op=mybir.AluOpType.add,
                        )
                        nc.vector.tensor_reduce(
                            out=out_ap,
                            in_=h2[:],
                            axis=mybir.AxisListType.X,
                            op=mybir.AluOpType.add,
                        )

                if w == nwaves // 2 + 4:
                    _emit_sq_half(nc, dist, sq, pacc, 0)

            _emit_sq_half(nc, dist, sq, pacc, 1)
            nc.sync.dma_start(
                out=pacc_d.ap(), in_=pacc[:].rearrange("p a b -> p (a b)")
            )
    nc.compile()
    return nc


def _get(name):
    if name not in _built:
        if name == "A":
            _built[name] = _build_launch_a()
        else:
            _built[name] = _build_launch_b()
    return _built[name]


def _prep_a(emb_flat, lab_flat):
    in_maps = []
    kk = np.arange(K, dtype=np.int32)
    for c in range(NCORES):
        e = emb_flat[c * NPIX : (c + 1) * NPIX].astype(FP8).reshape(P, TC * D)
        l = lab_flat[c * NPIX : (c + 1) * NPIX].reshape(P, TC)
        oh = (l[:, :, None] == kk[None, None, :]).astype(FP8)
        in_maps.append({"embA": e, "onehotA": oh.reshape(P, TC * K)})
    return in_maps


def _reduce_a(results, lab_flat):
    sums = np.zeros((B, K, D), dtype=np.float64)
    for c in range(NCORES):
        o = results[c]["outA"].astype(np.float64).reshape(P, NBANKS_A, 8, K)
        o = o.sum(axis=1)
        sbc = c // 2
        for j in range(8):
            sums[sbc] += o[D * j : D * j + D, j, :].T
    cnts = np.zeros((B, K), dtype=np.int64)
    spl = NPIX_TOT // B
    for b in range(B):
        cnts[b] = np.bincount(lab_flat[b * spl : (b + 1) * spl], minlength=K)
    cents = sums / np.maximum(cnts, 1)[:, :, None]
    cents = np.where(cnts[:, :, None] > 0, cents, 0.0)
    return cents, cnts


def _prep_b(emb_flat, lab_flat, cents):
    cb = cents.astype(F32)
    rtab = np.zeros((CR, 4 * D), dtype=BF16)
    rtab[:K, :] = cb.transpose(1, 0, 2).reshape(K, 4 * D).astype(BF16)
    eye = -np.eye(D, dtype=F32)
    for b in range(4):
        rtab[K:, b * D : (b + 1) * D] = eye.astype(BF16)

    in_maps = []
    kk = np.arange(K, dtype=np.int32)
    for c in range(NCORES):
        e = emb_flat[c * NPIX : (c + 1) * NPIX].astype(BF16).reshape(P, TC, D)
        l = lab_flat[c * NPIX : (c + 1) * NPIX].reshape(P, TC)
        ohT = (l.T[:, None, :] == kk[None, :, None]).astype(BF16)
        eT = np.ascontiguousarray(e.transpose(1, 2, 0))
        w_all = np.concatenate([ohT, eT], axis=1)
        wev = np.ascontiguousarray(w_all[:HT].transpose(1, 0, 2)).reshape(CR, HT * P)
        wod = np.ascontiguousarray(w_all[HT:].transpose(1, 0, 2)).reshape(CR, HT * P)
        in_maps.append({"wev": wev, "wod": wod, "rtab": rtab.copy()})
    return in_maps


def _push_host(cents):
    cb = cents.astype(np.float64)
    d = np.abs(cb[:, :, None, :] - cb[:, None, :, :]).sum(axis=-1)
    m = np.maximum(PUSH_MARGIN - d, 0.0)
    iu = np.triu(np.ones((K, K), dtype=bool), k=1)
    return (m * m * iu[None]).sum(axis=(1, 2)) / NCMP


def run_launches(embeddings, labels, trace=False, trace_kwargs=None):
    emb_flat = np.ascontiguousarray(np.asarray(embeddings), dtype=F32).reshape(
        NPIX_TOT, D
    )
    lab_flat = np.ascontiguousarray(np.asarray(labels), dtype=np.int32).reshape(
        NPIX_TOT
    )
    core_ids = list(range(NCORES))

    kwA = dict(trace=trace, **(trace_kwargs or {}))
    resA = run_bass_kernel_spmd(_get("A"), _prep_a(emb_flat, lab_flat), core_ids, **kwA)
    cents, _ = _reduce_a(resA.results, lab_flat)

    resB = run_bass_kernel_spmd(
        _get("B"), _prep_b(emb_flat, lab_flat, cents), core_ids, **kwA
    )
    pull = np.zeros(4, dtype=np.float64)
    for c in range(NCORES):
        pull += (
            resB.results[c]["pacc"].astype(np.float64).reshape(P, 2, 4).sum(axis=(0, 1))
        )
    pull /= NPIX_TOT

    push = _push_host(cents)

    loss = np.mean(PUSH_W * push + PULL_W * pull)
    return np.array(loss, dtype=F32), resA, resB


def kernel(embeddings, labels):
    loss, _, _ = run_launches(embeddings, labels, trace=False)
    return loss


# revision 27
# speedup vs baseline: 1.0527x; 1.0527x over previous
import numpy as np
import ml_dtypes

import concourse.bass as bass
import concourse.bacc as bacc
import concourse.mybir as mybir
from concourse.tile import TileContext
from concourse.bass_utils import run_bass_kernel_spmd

BF16 = ml_dtypes.bfloat16
FP8 = ml_dtypes.float8_e4m3
F32 = np.float32

B, H, W, D, K = 4, 384, 384, 16, 32
NCORES = 8
NPIX_TOT = B * H * W
NPIX = NPIX_TOT // NCORES
P = 128
TC = NPIX // P
NBANKS_A = 8

HT = TC // 2
CR = 48
WT = 16

PUSH_MARGIN = 0.25
PUSH_W = 1.0
PULL_W = 0.1
NCMP = K * (K - 1) / 2.0

CONSUMER_PATTERN = "DDDDDDDDDDDDDDDDDD"

_built = {}


def _build_launch_a():
    nc = bacc.Bacc("TRN2", target_bir_lowering=False, debug=False)
    f8 = mybir.dt.float8e4
    f32 = mybir.dt.float32

    NGM = TC // 8
    embA = nc.dram_tensor("embA", [P, TC * D], f8, kind="ExternalInput")
    onehotA = nc.dram_tensor("onehotA", [P, TC * K], f8, kind="ExternalInput")
    outA = nc.dram_tensor("outA", [P, NBANKS_A, 8 * K], f32, kind="ExternalOutput")

    with TileContext(nc) as tc:
        with (
            tc.tile_pool(name="sbuf", bufs=1) as pool,
            tc.tile_pool(name="psum", bufs=1, space="PSUM") as psum_pool,
        ):
            emb_sb = pool.tile([P, TC * D], f8)
            onehot = pool.tile([P, TC, K], f8)

            NCH = 4
            ch = (TC * D) // NCH
            och = TC // NCH
            for i in range(NCH):
                nc.sync.dma_start(
                    out=emb_sb[:, i * ch : (i + 1) * ch],
                    in_=embA.ap()[:, i * ch : (i + 1) * ch],
                )
                nc.sync.dma_start(
                    out=onehot[:, i * och : (i + 1) * och, :],
                    in_=onehotA.ap().rearrange("p (t k) -> p t k", k=K)[
                        :, i * och : (i + 1) * och, :
                    ],
                )

            banks = [
                psum_pool.tile([P, 8, K], mybir.dt.float32, name=f"acc{b}")
                for b in range(NBANKS_A)
            ]
            for g in range(NGM):
                nc.tensor.matmul(
                    banks[g % NBANKS_A][:],
                    emb_sb[:, 128 * g : 128 * g + 128],
                    onehot[:, 8 * g : 8 * g + 8, :],
                    start=(g < NBANKS_A),
                    stop=(g >= NGM - NBANKS_A),
                )

            evac = pool.tile([P, NBANKS_A, 8 * K], f32)
            for b in range(NBANKS_A):
                if b % 2 == 0:
                    nc.scalar.copy(
                        out=evac[:, b, :],
                        in_=banks[b][:].rearrange("p a b -> p (a b)"),
                    )
                else:
                    nc.vector.tensor_copy(
                        out=evac[:, b, :],
                        in_=banks[b][:].rearrange("p a b -> p (a b)"),
                    )
            nc.sync.dma_start(out=outA.ap(), in_=evac[:])
    nc.compile()
    return nc


def _emit_sq_half(nc, dist, sq, pacc, h):
    HH = TC // 2
    sl = slice(h * HH, (h + 1) * HH)
    nc.vector.tensor_tensor(
        out=sq[:, sl, :], in0=dist[:, sl, :], in1=dist[:, sl, :],
        op=mybir.AluOpType.mult,
    )
    for b in range(4):
        nc.vector.tensor_reduce(
            out=pacc[:, h, b : b + 1],
            in_=sq[:, sl, b],
            axis=mybir.AxisListType.X,
            op=mybir.AluOpType.add,
        )


def _build_launch_b():
    nc = bacc.Bacc("TRN2", target_bir_lowering=False, debug=False)
    bf = mybir.dt.bfloat16
    f32 = mybir.dt.float32

    wev = nc.dram_tensor("wev", [CR, HT * P], bf, kind="ExternalInput")
    wod = nc.dram_tensor("wod", [CR, HT * P], bf, kind="ExternalInput")
    rtab = nc.dram_tensor("rtab", [CR, 4 * D], bf, kind="ExternalInput")
    pacc_d = nc.dram_tensor("pacc", [P, 8], f32, kind="ExternalOutput")

    AF = mybir.ActivationFunctionType

    with TileContext(nc) as tc:
        with (
            tc.tile_pool(name="sbuf", bufs=1) as pool,
            tc.tile_pool(name="work", bufs=3) as wpool,
            tc.tile_pool(name="psum", bufs=2, space="PSUM") as psum_pool,
        ):
            w_sb = pool.tile([P, HT, P], bf)
            rhs_sb = pool.tile([P, 4 * D], bf)
            dist = pool.tile([P, TC, 4], bf)
            sq = pool.tile([P, TC, 4], bf)
            pacc = pool.tile([P, 2, 4], f32)

            nc.sync.dma_start(out=rhs_sb[0:CR, :], in_=rtab.ap())
            nc.sync.dma_start(out=rhs_sb[64 : 64 + CR, :], in_=rtab.ap())
            bounds = [0, 32, 96, 160, 224, HT]
            for i in range(len(bounds) - 1):
                sl = slice(bounds[i], bounds[i + 1])
                nc.sync.dma_start(
                    out=w_sb[0:CR, sl, :],
                    in_=wev.ap().rearrange("r (t m) -> r t m", m=P)[:, sl, :],
                )
                nc.sync.dma_start(
                    out=w_sb[64 : 64 + CR, sl, :],
                    in_=wod.ap().rearrange("r (t m) -> r t m", m=P)[:, sl, :],
                )

            nwaves = HT // WT
            for w in range(nwaves):
                t0 = WT * w
                ps = psum_pool.tile(
                    [P, 2, WT, 4, D], mybir.dt.float32, tag="ps", name=f"ps_{w}"
                )
                for j in range(WT):
                    t = t0 + j
                    nc.tensor.matmul(
                        ps[:, 0, j, :, :].rearrange("p a b -> p (a b)"),
                        w_sb[0:CR, t, :],
                        rhs_sb[0:CR, :],
                        start=True,
                        stop=True,
                    )
                    nc.tensor.matmul(
                        ps[:, 1, j, :, :].rearrange("p a b -> p (a b)"),
                        w_sb[64 : 64 + CR, t, :],
                        rhs_sb[64 : 64 + CR, :],
                        start=True,
                        stop=True,
                    )
                kind = CONSUMER_PATTERN[w % len(CONSUMER_PATTERN)]
                out_ap = dist[:, 2 * t0 : 2 * t0 + 2 * WT, :].rearrange(
                    "p (h t) b -> p h t b", h=2
                )
                with nc.allow_low_precision("dist bf16; error averages out"):
                    if kind == "D":
                        nc.vector.tensor_reduce(
                            out=out_ap,
                            in_=ps[:],
                            axis=mybir.AxisListType.X,
                            op=mybir.AluOpType.add,
                            apply_absolute_value=True,
                        )
                    else:
                        absd = wpool.tile([P, 2, WT, 4, D], bf, tag="absd")
                        nc.scalar.activation(out=absd[:], in_=ps[:], func=AF.Abs)
                        h1 = wpool.tile([P, 2, WT, 4, D // 2], bf, tag="h1")
                        h1_eng = nc.gpsimd if kind == "P" else nc.vector
                        h1_eng.tensor_tensor(
                            out=h1[:],
                            in0=absd[:, :, :, :, 0 : D // 2],
                            in1=absd[:, :, :, :, D // 2 : D],
                            op=mybir.AluOpType.add,
                        )
                        h2 = wpool.tile([P, 2, WT, 4, D // 4], bf, tag="h2")
                        nc.vector.tensor_tensor(
                            out=h2[:],
                            in0=h1[:, :, :, :, 0 : D // 4],
                            in1=h1[:, :, :, :, D // 4 : D // 2],
                            op=mybir.AluOpType.add,
                        )
                        nc.vector.tensor_reduce(
                            out=out_ap,
                            in_=h2[:],
                            axis=mybir.AxisListType.X,
                            op=mybir.AluOpType.add,
                        )

                if w == nwaves // 2 + 4:
                    _emit_sq_half(nc, dist, sq, pacc, 0)

            _emit_sq_half(nc, dist, sq, pacc, 1)
            nc.sync.dma_start(
                out=pacc_d.ap(), in_=pacc[:].rearrange("p a b -> p (a b)")
            )
    nc.compile()
    return nc


def _get(name):
    if name not in _built:
        if name == "A":
            _built[name] = _build_launch_a()
        else:
            _built[name] = _build_launch_b()
    return _built[name]


def _prep_a(emb_flat, lab_flat):
    in_maps = []
    kk = np.arange(K, dtype=np.int32)
    for c in range(NCORES):
        e = emb_flat[c * NPIX : (c + 1) * NPIX].astype(FP8).reshape(P, TC * D)
        l = lab_flat[c * NPIX : (c + 1) * NPIX].reshape(P, TC)
        oh = (l[:, :, None] == kk[None, None, :]).astype(FP8)
        in_maps.append({"embA": e, "onehotA": oh.reshape(P, TC * K)})
    return in_maps


def _reduce_a(results, lab_flat):
    sums = np.zeros((B, K, D), dtype=np.float64)
    for c in range(NCORES):
        o = results[c]["outA"].astype(np.float64).reshape(P, NBANKS_A, 8, K)
        o = o.sum(axis=1)
        sbc = c // 2
        for j in range(8):
            sums[sbc] += o[D * j : D * j + D, j, :].T
    cnts = np.zeros((B, K), dtype=np.int64)
    spl = NPIX_TOT // B
    for b in range(B):
        cnts[b] = np.bincount(lab_flat[b * spl : (b + 1) * spl], minlength=K)
    cents = sums / np.maximum(cnts, 1)[:, :, None]
    cents = np.where(cnts[:, :, None] > 0, cents, 0.0)
    return cents, cnts


def _prep_b(emb_flat, lab_flat, cents):
    cb = cents.astype(F32)
    rtab = np.zeros((CR, 4 * D), dtype=BF16)
    rtab[:K, :] = cb.transpose(1, 0, 2).reshape(K, 4 * D).astype(BF16)
    eye = -np.eye(D, dtype=F32)
    for b in range(4):
        rtab[K:, b * D : (b + 1) * D] = eye.astype(BF16)

    in_maps = []
    kk = np.arange(K, dtype=np.int32)
    for c in range(NCORES):
        e = emb_flat[c * NPIX : (c + 1) * NPIX].astype(BF16).reshape(P, TC, D)
        l = lab_flat[c * NPIX : (c + 1) * NPIX].reshape(P, TC)
        ohT = (l.T[:, None, :] == kk[None, :, None]).astype(BF16)
        eT = np.ascontiguousarray(e.transpose(1, 2, 0))
        w_all = np.concatenate([ohT, eT], axis=1)
        wev = np.ascontiguousarray(w_all[:HT].transpose(1, 0, 2)).reshape(CR, HT * P)
        wod = np.ascontiguousarray(w_all[HT:].transpose(1, 0, 2)).reshape(CR, HT * P)
        in_maps.append({"wev": wev, "wod": wod, "rtab": rtab.copy()})
    return in_maps


def _push_host(cents):
    cb = cents.astype(np.float64)
    d = np.abs(cb[:, :, None, :] - cb[:, None, :, :]).sum(axis=-1)
    m = np.maximum(PUSH_MARGIN - d, 0.0)
    iu = np.triu(np.ones((K, K), dtype=bool), k=1)
    return (m * m * iu[None]).sum(axis=(1, 2)) / NCMP


def run_launches(embeddings, labels, trace=False, trace_kwargs=None):
    emb_flat = np.ascontiguousarray(np.asarray(embeddings), dtype=F32).reshape(
        NPIX_TOT, D
    )
    lab_flat = np.ascontiguousarray(np.asarray(labels), dtype=np.int32).reshape(
        NPIX_TOT
    )
    core_ids = list(range(NCORES))

    kwA = dict(trace=trace, **(trace_kwargs or {}))
    resA = run_bass_kernel_spmd(_get("A"), _prep_a(emb_flat, lab_flat), core_ids, **kwA)
    cents, _ = _reduce_a(resA.results, lab_flat)

    resB = run_bass_kernel_spmd(
        _get("B"), _prep_b(emb_flat, lab_flat, cents), core_ids, **kwA
    )
    pull = np.zeros(4, dtype=np.float64)
    for c in range(NCORES):
        pull += (
            resB.results[c]["pacc"].astype(np.float64).reshape(P, 2, 4).sum(axis=(0, 1))
        )
    pull /= NPIX_TOT

    push = _push_host(cents)

    loss = np.mean(PUSH_W * push + PULL_W * pull)
    return np.array(loss, dtype=F32), resA, resB


def kernel(embeddings, labels):
    loss, _, _ = run_launches(embeddings, labels, trace=False)
    return loss


# revision 28
# speedup vs baseline: 1.1461x; 1.0887x over previous
import numpy as np
import ml_dtypes

import concourse.bass as bass
import concourse.bacc as bacc
import concourse.mybir as mybir
from concourse.tile import TileContext
from concourse.bass_utils import run_bass_kernel_spmd

BF16 = ml_dtypes.bfloat16
FP8 = ml_dtypes.float8_e4m3
F32 = np.float32

B, H, W, D, K = 4, 384, 384, 16, 32
NCORES = 8
NPIX_TOT = B * H * W
NPIX = NPIX_TOT // NCORES
P = 128
TC = NPIX // P
NBANKS_A = 8

HT = TC // 2
CR = 48
WT = 16

PUSH_MARGIN = 0.25
PUSH_W = 1.0
PULL_W = 0.1
NCMP = K * (K - 1) / 2.0

CONSUMER_PATTERN = "AAAAADAAAAADAAAAAA"

_built = {}


def _build_launch_a():
    nc = bacc.Bacc("TRN2", target_bir_lowering=False, debug=False)
    f8 = mybir.dt.float8e4
    f32 = mybir.dt.float32

    NGM = TC // 8
    embA = nc.dram_tensor("embA", [P, TC * D], f8, kind="ExternalInput")
    onehotA = nc.dram_tensor("onehotA", [P, TC * K], f8, kind="ExternalInput")
    outA = nc.dram_tensor("outA", [P, NBANKS_A, 8 * K], f32, kind="ExternalOutput")

    with TileContext(nc) as tc:
        with (
            tc.tile_pool(name="sbuf", bufs=1) as pool,
            tc.tile_pool(name="psum", bufs=1, space="PSUM") as psum_pool,
        ):
            emb_sb = pool.tile([P, TC * D], f8)
            onehot = pool.tile([P, TC, K], f8)

            NCH = 4
            ch = (TC * D) // NCH
            och = TC // NCH
            for i in range(NCH):
                nc.sync.dma_start(
                    out=emb_sb[:, i * ch : (i + 1) * ch],
                    in_=embA.ap()[:, i * ch : (i + 1) * ch],
                )
                nc.sync.dma_start(
                    out=onehot[:, i * och : (i + 1) * och, :],
                    in_=onehotA.ap().rearrange("p (t k) -> p t k", k=K)[
                        :, i * och : (i + 1) * och, :
                    ],
                )

            banks = [
                psum_pool.tile([P, 8, K], mybir.dt.float32, name=f"acc{b}")
                for b in range(NBANKS_A)
            ]
            for g in range(NGM):
                nc.tensor.matmul(
                    banks[g % NBANKS_A][:],
                    emb_sb[:, 128 * g : 128 * g + 128],
                    onehot[:, 8 * g : 8 * g + 8, :],
                    start=(g < NBANKS_A),
                    stop=(g >= NGM - NBANKS_A),
                )

            evac = pool.tile([P, NBANKS_A, 8 * K], f32)
            for b in range(NBANKS_A):
                if b % 2 == 0:
                    nc.scalar.copy(
                        out=evac[:, b, :],
                        in_=banks[b][:].rearrange("p a b -> p (a b)"),
                    )
                else:
                    nc.vector.tensor_copy(
                        out=evac[:, b, :],
                        in_=banks[b][:].rearrange("p a b -> p (a b)"),
                    )
            nc.sync.dma_start(out=outA.ap(), in_=evac[:])
    nc.compile()
    return nc


def _emit_batch_reduce(nc, absd_all, dist, g):
    a4 = absd_all[:, 4 * g : 4 * g + 4]
    with nc.allow_low_precision("dist bf16; error averages out"):
        nc.vector.tensor_tensor(
            out=a4[:, :, :, :, :, 0 : D // 2],
            in0=a4[:, :, :, :, :, 0 : D // 2],
            in1=a4[:, :, :, :, :, D // 2 : D],
            op=mybir.AluOpType.add,
        )
        nc.vector.tensor_tensor(
            out=a4[:, :, :, :, :, 0 : D // 4],
            in0=a4[:, :, :, :, :, 0 : D // 4],
            in1=a4[:, :, :, :, :, D // 4 : D // 2],
            op=mybir.AluOpType.add,
        )
        nc.vector.tensor_reduce(
            out=dist[:, 512 * g // 4 : 512 * g // 4 + 128, :].rearrange(
                "p (w h t) b -> p w h t b", w=4, h=2
            ),
            in_=a4[:, :, :, :, :, 0 : D // 4],
            axis=mybir.AxisListType.X,
            op=mybir.AluOpType.add,
        )


def _emit_sq_half(nc, dist, sq, pacc, h):
    HH = TC // 2
    sl = slice(h * HH, (h + 1) * HH)
    nc.vector.tensor_tensor(
        out=sq[:, sl, :], in0=dist[:, sl, :], in1=dist[:, sl, :],
        op=mybir.AluOpType.mult,
    )
    for b in range(4):
        nc.vector.tensor_reduce(
            out=pacc[:, h, b : b + 1],
            in_=sq[:, sl, b],
            axis=mybir.AxisListType.X,
            op=mybir.AluOpType.add,
        )


def _build_launch_b():
    nc = bacc.Bacc("TRN2", target_bir_lowering=False, debug=False)
    bf = mybir.dt.bfloat16
    f32 = mybir.dt.float32

    wev = nc.dram_tensor("wev", [CR, HT * P], bf, kind="ExternalInput")
    wod = nc.dram_tensor("wod", [CR, HT * P], bf, kind="ExternalInput")
    rtab = nc.dram_tensor("rtab", [CR, 4 * D], bf, kind="ExternalInput")
    pacc_d = nc.dram_tensor("pacc", [P, 8], f32, kind="ExternalOutput")

    AF = mybir.ActivationFunctionType

    with TileContext(nc) as tc:
        with (
            tc.tile_pool(name="sbuf", bufs=1) as pool,
            tc.tile_pool(name="work", bufs=3) as wpool,
            tc.tile_pool(name="psum", bufs=2, space="PSUM") as psum_pool,
        ):
            w_sb = pool.tile([P, HT, P], bf)
            rhs_sb = pool.tile([P, 4 * D], bf)
            dist = pool.tile([P, TC, 4], bf)
            sq = pool.tile([P, TC, 4], bf)
            pacc = pool.tile([P, 2, 4], f32)

            nc.sync.dma_start(out=rhs_sb[0:CR, :], in_=rtab.ap())
            nc.sync.dma_start(out=rhs_sb[64 : 64 + CR, :], in_=rtab.ap())
            bounds = [0, 32, 96, 160, 224, HT]
            for i in range(len(bounds) - 1):
                sl = slice(bounds[i], bounds[i + 1])
                nc.sync.dma_start(
                    out=w_sb[0:CR, sl, :],
                    in_=wev.ap().rearrange("r (t m) -> r t m", m=P)[:, sl, :],
                )
                nc.sync.dma_start(
                    out=w_sb[64 : 64 + CR, sl, :],
                    in_=wod.ap().rearrange("r (t m) -> r t m", m=P)[:, sl, :],
                )

            NA = CONSUMER_PATTERN.count("A")
            absd_all = pool.tile([P, NA, 2, WT, 4, D], bf)
            nwaves = HT // WT
            n_a = 0
            n_d = 0
            for w in range(nwaves):
                t0 = WT * w
                ps = psum_pool.tile(
                    [P, 2, WT, 4, D], mybir.dt.float32, tag="ps", name=f"ps_{w}"
                )
                for j in range(WT):
                    t = t0 + j
                    nc.tensor.matmul(
                        ps[:, 0, j, :, :].rearrange("p a b -> p (a b)"),
                        w_sb[0:CR, t, :],
                        rhs_sb[0:CR, :],
                        start=True,
                        stop=True,
                    )
                    nc.tensor.matmul(
                        ps[:, 1, j, :, :].rearrange("p a b -> p (a b)"),
                        w_sb[64 : 64 + CR, t, :],
                        rhs_sb[64 : 64 + CR, :],
                        start=True,
                        stop=True,
                    )
                kind = CONSUMER_PATTERN[w % len(CONSUMER_PATTERN)]
                with nc.allow_low_precision("dist bf16; error averages out"):
                    if kind == "D":
                        out_ap = dist[
                            :, 2 * WT * NA + 2 * WT * n_d : 2 * WT * NA + 2 * WT * (n_d + 1), :
                        ].rearrange("p (h t) b -> p h t b", h=2)
                        nc.vector.tensor_reduce(
                            out=out_ap,
                            in_=ps[:],
                            axis=mybir.AxisListType.X,
                            op=mybir.AluOpType.add,
                            apply_absolute_value=True,
                        )
                        n_d += 1
                    else:
                        nc.scalar.activation(
                            out=absd_all[:, n_a], in_=ps[:], func=AF.Abs
                        )
                        n_a += 1
                        if n_a % 4 == 0:
                            g = n_a // 4 - 1
                            _emit_batch_reduce(nc, absd_all, dist, g)
                if n_a == 12 and kind != "D":
                    _emit_sq_half(nc, dist, sq, pacc, 0)

            _emit_sq_half(nc, dist, sq, pacc, 1)
            nc.sync.dma_start(
                out=pacc_d.ap(), in_=pacc[:].rearrange("p a b -> p (a b)")
            )
    nc.compile()
    return nc


def _get(name):
    if name not in _built:
        if name == "A":
            _built[name] = _build_launch_a()
        else:
            _built[name] = _build_launch_b()
    return _built[name]


def _prep_a(emb_flat, lab_flat):
    in_maps = []
    kk = np.arange(K, dtype=np.int32)
    for c in range(NCORES):
        e = emb_flat[c * NPIX : (c + 1) * NPIX].astype(FP8).reshape(P, TC * D)
        l = lab_flat[c * NPIX : (c + 1) * NPIX].reshape(P, TC)
        oh = (l[:, :, None] == kk[None, None, :]).astype(FP8)
        in_maps.append({"embA": e, "onehotA": oh.reshape(P, TC * K)})
    return in_maps


def _reduce_a(results, lab_flat):
    sums = np.zeros((B, K, D), dtype=np.float64)
    for c in range(NCORES):
        o = results[c]["outA"].astype(np.float64).reshape(P, NBANKS_A, 8, K)
        o = o.sum(axis=1)
        sbc = c // 2
        for j in range(8):
            sums[sbc] += o[D * j : D * j + D, j, :].T
    cnts = np.zeros((B, K), dtype=np.int64)
    spl = NPIX_TOT // B
    for b in range(B):
        cnts[b] = np.bincount(lab_flat[b * spl : (b + 1) * spl], minlength=K)
    cents = sums / np.maximum(cnts, 1)[:, :, None]
    cents = np.where(cnts[:, :, None] > 0, cents, 0.0)
    return cents, cnts


def _prep_b(emb_flat, lab_flat, cents):
    cb = cents.astype(F32)
    rtab = np.zeros((CR, 4 * D), dtype=BF16)
    rtab[:K, :] = cb.transpose(1, 0, 2).reshape(K, 4 * D).astype(BF16)
    eye = -np.eye(D, dtype=F32)
    for b in range(4):
        rtab[K:, b * D : (b + 1) * D] = eye.astype(BF16)

    in_maps = []
    kk = np.arange(K, dtype=np.int32)
    for c in range(NCORES):
        e = emb_flat[c * NPIX : (c + 1) * NPIX].astype(BF16).reshape(P, TC, D)
        l = lab_flat[c * NPIX : (c + 1) * NPIX].reshape(P, TC)
        ohT = (l.T[:, None, :] == kk[None, :, None]).astype(BF16)
        eT = np.ascontiguousarray(e.transpose(1, 2, 0))
        w_all = np.concatenate([ohT, eT], axis=1)
        wev = np.ascontiguousarray(w_all[:HT].transpose(1, 0, 2)).reshape(CR, HT * P)
        wod = np.ascontiguousarray(w_all[HT:].transpose(1, 0, 2)).reshape(CR, HT * P)
        in_maps.append({"wev": wev, "wod": wod, "rtab": rtab.copy()})
    return in_maps


def _push_host(cents):
    cb = cents.astype(np.float64)
    d = np.abs(cb[:, :, None, :] - cb[:, None, :, :]).sum(axis=-1)
    m = np.maximum(PUSH_MARGIN - d, 0.0)
    iu = np.triu(np.ones((K, K), dtype=bool), k=1)
    return (m * m * iu[None]).sum(axis=(1, 2)) / NCMP


def run_launches(embeddings, labels, trace=False, trace_kwargs=None):
    emb_flat = np.ascontiguousarray(np.asarray(embeddings), dtype=F32).reshape(
        NPIX_TOT, D
    )
    lab_flat = np.ascontiguousarray(np.asarray(labels), dtype=np.int32).reshape(
        NPIX_TOT
    )
    core_ids = list(range(NCORES))

    kwA = dict(trace=trace, **(trace_kwargs or {}))
    resA = run_bass_kernel_spmd(_get("A"), _prep_a(emb_flat, lab_flat), core_ids, **kwA)
    cents, _ = _reduce_a(resA.results, lab_flat)

    resB = run_bass_kernel_spmd(
        _get("B"), _prep_b(emb_flat, lab_flat, cents), core_ids, **kwA
    )
    pull = np.zeros(4, dtype=np.float64)
    for c in range(NCORES):
        pull += (
            resB.results[c]["pacc"].astype(np.float64).reshape(P, 2, 4).sum(axis=(0, 1))
        )
    pull /= NPIX_TOT

    push = _push_host(cents)

    loss = np.mean(PUSH_W * push + PULL_W * pull)
    return np.array(loss, dtype=F32), resA, resB


def kernel(embeddings, labels):
    loss, _, _ = run_launches(embeddings, labels, trace=False)
    return loss
